# revision 1
# baseline (speedup 1.0000x reference)
"""DGI (Deep Graph Infomax) kernel for 8 Trainium2 NeuronCores.

Strategy (row-wise graph partitioning, per the sharding hint):
  - nodes split 12500/core (padded to 12544 = 98*128 rows); each core owns the
    incoming edges of its node block.
  - phase 1: each core computes its shard of xtheta = x @ W^T + b for both
    graphs directly in row layout (lhsT = x-chunk, rhs = W^T; no PE
    transposes), bf16 inputs, rows stored as [node, pos_h(64) | neg_h(64)]
    bf16 (256B rows); AllGather -> full 100352-row table in HBM.
  - per-edge gather of the 256B source rows via gpsimd dma_gather (int16
    indices => table processed as 4 buckets of 25088 rows). The 4 gather
    streams are spread over 4 SWDGE queues (num_swdge_queues=4), which
    pipelines descriptor processing ~4x vs the single-queue default
    (measured 1.8 ns/row vs 7.4 ns/row).
  - segment-sum via matmul: for each group of 128 destination rows, the
    gathered 128-edge blocks (lhsT, [e, h2]) are multiplied by a one-hot
    selection matrix M[e, r] = v_e * (row_e == r) built on DVE with one
    tensor_scalar(is_equal, mult), accumulating agg^T[h2, r] in PSUM.
  - PReLU + mean-readout row-sum fused into ONE Activation-engine op
    (activation(Prelu, alpha=AP, accum_out=...)), keeping DVE free for the
    one-hot builds.
  - mean over nodes via AllReduce, sigmoid, z = W_bil @ s, then per-tile
    matmuls score[n,{pos,neg}] = H^T[:, n-tile].T @ [z|0 , 0|z] in bf16.
"""

import sys

try:
    import concourse.bacc as bacc
except ImportError:  # pragma: no cover
    sys.path.insert(0, "/opt/trn_rl_repo")
    import concourse.bacc as bacc

import numpy as np
import ml_dtypes

import concourse.bass as bass
import concourse.mybir as mybir
import concourse.tile as tile
from concourse.library_config import mlp
from concourse.bass_utils import run_bass_kernel_spmd

P = 128
BF16 = mybir.dt.bfloat16
F32 = mybir.dt.float32
I16 = mybir.dt.int16

_NC_CACHE = {}


# --------------------------------------------------------------------------
# host-side planning
# --------------------------------------------------------------------------

class Plan:
    pass


def make_plan(n_nodes, ncores, edge_rows, edge_cols, edge_vals, g_chunk=7):
    """Static (shared-across-cores) schedule + per-core edge data arrays."""
    pl = Plan()
    local_n = n_nodes // ncores
    assert local_n * ncores == n_nodes
    local_pad = ((local_n + P - 1) // P) * P
    groups = local_pad // P
    n_buckets = 4
    assert ncores % n_buckets == 0
    ranks_per_bucket = ncores // n_buckets
    bucket_rows = ranks_per_bucket * local_pad
    assert bucket_rows <= 32767, bucket_rows

    pl.ncores, pl.local_n, pl.local_pad = ncores, local_n, local_pad
    pl.groups, pl.n_buckets, pl.bucket_rows = groups, n_buckets, bucket_rows
    pl.trows = ncores * local_pad

    r = np.asarray(edge_rows).astype(np.int64)
    c = np.asarray(edge_cols).astype(np.int64)
    v = np.asarray(edge_vals).astype(np.float32)

    core = r // local_n
    lr = r % local_n
    g = lr // P
    rloc = lr % P
    crank = c // local_n
    cloc = c % local_n
    q = crank // (2 if ranks_per_bucket == 2 else ranks_per_bucket)
    q = crank // ranks_per_bucket
    idx16 = (crank % ranks_per_bucket) * local_pad + cloc

    # per (core, g, q) segment counts
    key = (core * groups + g) * n_buckets + q
    counts = np.bincount(key, minlength=ncores * groups * n_buckets).reshape(
        ncores, groups, n_buckets
    )
    nblk = np.ceil(counts.max(axis=0) / P).astype(np.int64)  # [groups, n_buckets]
    # every group needs at least one block so its PSUM tile gets written
    empty_g = nblk.sum(axis=1) == 0
    nblk[empty_g, 0] = 1
    pl.nblk = nblk

    # chunk structure
    chunk_ids = [list(range(k, min(k + g_chunk, groups))) for k in range(0, groups, g_chunk)]
    pl.chunks = []
    jglobal = 0
    idx_off = 0  # in int16 free-columns of the [128, *] gidx tensor
    for chunk in chunk_ids:
        spec = Plan()
        spec.groups = chunk
        spec.idx_off = idx_off
        spec.nq = []
        spec.q_off = []   # offset inside this chunk's idx tile, int16 cols
        spec.blocks = {gg: [] for gg in chunk}
        qo = 0
        for qq in range(n_buckets):
            nq = int(sum(nblk[gg][qq] for gg in chunk))
            spec.nq.append(nq)
            spec.q_off.append(qo)
            pos = 0
            for gg in chunk:
                for _ in range(int(nblk[gg][qq])):
                    spec.blocks[gg].append((qq, pos, jglobal))
                    pos += 1
                    jglobal += 1
            qo += nq * 8  # nq*128 idxs -> /16 cols
        spec.idx_len = qo
        idx_off += qo
        pl.chunks.append(spec)
    pl.b_total = jglobal
    pl.gidx_cols = idx_off

    # ---- per-core data arrays ----
    # sort edges by (core, g, q, idx16) so segments are contiguous
    order = np.lexsort((idx16, q, g, core))
    so_core, so_g, so_q = core[order], g[order], q[order]
    so_idx, so_rloc, so_v = idx16[order], rloc[order], v[order]
    seg_key = ((so_core * groups + so_g) * n_buckets + so_q)
    seg_counts = np.bincount(seg_key, minlength=ncores * groups * n_buckets)
    seg_starts = np.concatenate([[0], np.cumsum(seg_counts)])

    pl.gidx = []
    pl.rl = []
    pl.vv = []
    for cc in range(ncores):
        all_idx = np.zeros(pl.b_total * P, np.int64)
        all_rloc = np.zeros(pl.b_total * P, np.float32)
        all_v = np.zeros(pl.b_total * P, np.float32)
        wpos = 0
        for spec in pl.chunks:
            for qq in range(n_buckets):
                for gg in spec.groups:
                    sk = (cc * groups + gg) * n_buckets + qq
                    s0, s1 = seg_starts[sk], seg_starts[sk + 1]
                    cnt = s1 - s0
                    slots = int(nblk[gg][qq]) * P
                    assert cnt <= slots
                    all_idx[wpos:wpos + cnt] = so_idx[s0:s1]
                    all_rloc[wpos:wpos + cnt] = so_rloc[s0:s1]
                    all_v[wpos:wpos + cnt] = so_v[s0:s1]
                    wpos += slots
        assert wpos == pl.b_total * P
        # wrap idx per gather call
        wrapped = []
        for spec in pl.chunks:
            base = 0
            for qq in range(n_buckets):
                nq = spec.nq[qq]
                if nq == 0:
                    continue
        # call boundaries: iterate chunks/q again tracking global edge pos
        pos = 0
        for spec in pl.chunks:
            for qq in range(n_buckets):
                nidx = spec.nq[qq] * P
                if nidx == 0:
                    continue
                sl = all_idx[pos:pos + nidx]
                w = sl.reshape(nidx // 16, 16).T.astype(np.int16)  # [16, nidx/16]
                wrapped.append(np.tile(w, (8, 1)))
                pos += nidx
        assert pos == pl.b_total * P
        gidx = np.concatenate(wrapped, axis=1)
        assert gidx.shape == (P, pl.gidx_cols)
        pl.gidx.append(np.ascontiguousarray(gidx))
        pl.rl.append(np.ascontiguousarray(
            all_rloc.reshape(pl.b_total, P).T.astype(np.float32)))
        pl.vv.append(np.ascontiguousarray(
            all_v.reshape(pl.b_total, P).T
            .astype(ml_dtypes.bfloat16).astype(np.float32)))
    return pl


# --------------------------------------------------------------------------
# device kernel build
# --------------------------------------------------------------------------

class _EarlyStop(Exception):
    pass


def _early_out(nc, tc, scores_d, groups):
    with tc.tile_pool(name="eo", bufs=1) as eo:
        scr = eo.tile([P, 2 * groups], F32)
        nc.vector.memset(scr[:], 0.0)
        nc.sync.dma_start(scores_d[:], scr[:])


def build_nc(pl, stop_after=None, timing_variant=False, repeat=1,
             gd_bufs=2, mb_bufs=4, pg_bufs=2, preload_idx=True,
             local_table=False, preload_x=True, mb_host=True):
    ncores, local_pad, groups = pl.ncores, pl.local_pad, pl.groups
    BR, trows = pl.bucket_rows, pl.trows
    stops = {"lin": 0, "ag": 1, "gatheronly": 2, "mbuild": 2.2, "mm": 2.5, "p2a": 2.8, "p2b": 2.9, "phase2": 3}
    level = stops.get(stop_after, 99)

    nc = bacc.Bacc("TRN2", target_bir_lowering=False, debug=False,
                   num_devices=ncores, enable_asserts=False,
                   num_swdge_queues=4)

    # inputs
    x2 = nc.dram_tensor("x2", [P, 2 * local_pad], BF16, kind="ExternalInput")
    w2 = nc.dram_tensor("w2", [P, 64], BF16, kind="ExternalInput")
    bias2 = nc.dram_tensor("bias2", [P, P], BF16, kind="ExternalInput")
    wbt = nc.dram_tensor("wbt", [64, 64], F32, kind="ExternalInput")
    acol = nc.dram_tensor("acol", [P, 1], F32, kind="ExternalInput")
    bbcol = nc.dram_tensor("bbcol", [P, 1], F32, kind="ExternalInput")
    iotab = nc.dram_tensor("iotab", [P, P], BF16, kind="ExternalInput")
    gidx_d = nc.dram_tensor("gidx", [P, pl.gidx_cols], I16, kind="ExternalInput")
    mbh_d = nc.dram_tensor("mbh", [P, pl.b_total * P], BF16,
                           kind="ExternalInput")
    rl_d = nc.dram_tensor("rl", [P, pl.b_total], F32, kind="ExternalInput")
    vv_d = nc.dram_tensor("vv", [P, pl.b_total], F32, kind="ExternalInput")

    scores_d = nc.dram_tensor("scores", [P, 2 * groups], F32, kind="ExternalOutput")

    # internal DRAM
    xt_c = nc.dram_tensor("xt_c", [local_pad, P], BF16)
    if timing_variant:
        # collective-free build for chained timing runs: the gather table is
        # supplied directly as an input, AllReduce becomes a local copy
        xt_all = nc.dram_tensor("xt_fake", [trows, P], BF16,
                                kind="ExternalInput")
        ar_in = nc.dram_tensor("ar_in", [64, 1], F32)
        ar_out = nc.dram_tensor("ar_out", [64, 1], F32)
    else:
        xt_all = nc.dram_tensor("xt_all", [trows, P], BF16, addr_space="Shared")
        ar_in = nc.dram_tensor("ar_in", [64, 1], F32)
        ar_out = nc.dram_tensor("ar_out", [64, 1], F32, addr_space="Shared")
    xt_loc = None
    if local_table and not timing_variant:
        # gathers from Shared-space DRAM can be slower than from regular
        # DRAM; stream the gathered table into a local copy first
        xt_loc = nc.dram_tensor("xt_loc", [trows, P], BF16)

    rg = [list(range(ncores))]
    inv_n = 1.0 / float(pl.local_n * ncores)

    with tile.TileContext(nc) as tc:
        nc.gpsimd.load_library(mlp)
        with (
            tc.tile_pool(name="const", bufs=1) as cpool,
            tc.tile_pool(name="big", bufs=1) as bigpool,
        ):
            w2_sb = cpool.tile([P, 64], BF16)
            nc.sync.dma_start(w2_sb[:], w2[:])
            bias2_sb = cpool.tile([P, P], BF16)
            nc.sync.dma_start(bias2_sb[:], bias2[:])
            wbt_sb = cpool.tile([64, 64], F32)
            nc.sync.dma_start(wbt_sb[:], wbt[:])
            a_sb = cpool.tile([P, 1], F32)
            nc.sync.dma_start(a_sb[:], acol[:])
            bb_sb = cpool.tile([P, 1], F32)
            nc.sync.dma_start(bb_sb[:], bbcol[:])
            iota_sb = cpool.tile([P, P], BF16)
            nc.sync.dma_start(iota_sb[:], iotab[:])
            rl_sb = bigpool.tile([P, pl.b_total], F32)
            nc.sync.dma_start(rl_sb[:], rl_d[:])
            vv_sb = bigpool.tile([P, pl.b_total], F32)
            nc.sync.dma_start(vv_sb[:], vv_d[:])

            for _rep in range(repeat):
                HT = bigpool.tile([P, local_pad], BF16, tag="HT")
                acc = bigpool.tile([P, groups], F32, tag="acc")

                # ---------------- phase 1: linear (row-major, no transpose) ----
                with (
                    tc.tile_pool(name="lin", bufs=3) as lpool,
                    tc.tile_pool(name="lpsum", bufs=2, space="PSUM") as lpsum,
                ):
                    x2_sb = None
                    if preload_x:
                        x2_sb = lpool.tile([P, 2 * local_pad], BF16,
                                           tag="x2full")
                        nc.sync.dma_start(x2_sb[:], x2[:])
                    for t in range(groups):
                        sl = slice(t * P, (t + 1) * P)
                        if preload_x:
                            lp = x2_sb[:, sl]
                            ln = x2_sb[:, local_pad + t * P:
                                       local_pad + (t + 1) * P]
                        else:
                            xin = lpool.tile([P, 256], BF16, tag="xin")
                            nc.sync.dma_start(xin[:, 0:128], x2[:, sl])
                            nc.sync.dma_start(
                                xin[:, 128:256],
                                x2[:, local_pad + t * P:local_pad + (t + 1) * P])
                            lp = xin[:, 0:128]
                            ln = xin[:, 128:256]
                        pt = lpsum.tile([P, 128], F32, tag="pt")
                        nc.tensor.matmul(pt[:, 0:64], lhsT=lp,
                                         rhs=w2_sb[:], start=True, stop=True)
                        nc.tensor.matmul(pt[:, 64:128], lhsT=ln,
                                         rhs=w2_sb[:], start=True, stop=True)
                        xrow = lpool.tile([P, 128], BF16, tag="xrow")
                        nc.vector.scalar_tensor_tensor(
                            xrow[:], pt[:], 1.0, bias2_sb[:],
                            mybir.AluOpType.mult, mybir.AluOpType.add)
                        nc.sync.dma_start(xt_c[sl, :], xrow[:])

                # ---------------- all-gather xtheta ----------------------------
                if level >= 1 and not timing_variant:
                    nc.gpsimd.collective_compute(
                        "AllGather", mybir.AluOpType.bypass, replica_groups=rg,
                        ins=[xt_c.ap().opt()], outs=[xt_all.ap().opt()],
                    )

                # ---------------- phase 2: gather + segment-sum matmul ---------
                if level >= 2 and xt_loc is not None:
                    nc.sync.dma_start(xt_loc[:, :], xt_all[:, :])
                gtab = xt_loc if xt_loc is not None else xt_all
                if level >= 2:
                    with (
                        tc.tile_pool(name="gath", bufs=gd_bufs) as gpool,
                        tc.tile_pool(name="idxp", bufs=2) as ipool,
                        tc.tile_pool(name="mb", bufs=mb_bufs) as mpool,
                        tc.tile_pool(name="mbst", bufs=2) as mspool,
                        tc.tile_pool(name="gpsum", bufs=pg_bufs, space="PSUM") as gpsum,
                    ):
                        idx_full = None
                        if preload_idx:
                            idx_full = ipool.tile([P, pl.gidx_cols], I16,
                                                  tag="idxfull")
                            nc.sync.dma_start(idx_full[:], gidx_d[:])
                        for spec in pl.chunks:
                            nbc = sum(len(spec.blocks[gg]) for gg in spec.groups)
                            j0 = min(j for gg in spec.groups
                                     for (_, _, j) in spec.blocks[gg])
                            mb_sb = None
                            if mb_host:
                                mb_sb = mspool.tile([P, nbc * P], BF16,
                                                    tag="mbs")
                                nc.sync.dma_start(
                                    mb_sb[:],
                                    mbh_d[:, j0 * P:(j0 + nbc) * P])
                            if preload_idx:
                                idxt = idx_full[:,
                                                spec.idx_off:spec.idx_off + spec.idx_len]
                            else:
                                idxt_t = ipool.tile([P, spec.idx_len], I16, tag="idx")
                                nc.sync.dma_start(
                                    idxt_t[:],
                                    gidx_d[:, spec.idx_off:spec.idx_off + spec.idx_len])
                                idxt = idxt_t[:]
                            gds = {}
                            for qq in range(pl.n_buckets):
                                nq = spec.nq[qq]
                                if nq == 0:
                                    continue
                                gd = gpool.tile([P, nq, P], BF16, tag=f"gd{qq}")
                                nidx = nq * P
                                qo = spec.q_off[qq]
                                nc.gpsimd.dma_gather(
                                    gd[:], gtab[qq * BR:(qq + 1) * BR, :],
                                    idxt[:, qo:qo + nidx // 16], nidx, nidx, P,
                                    single_packet=(nidx <= 1024),
                                    queue_num=qq,
                                )
                                gds[qq] = gd
                            if level < 3:
                                continue
                            for gg in spec.groups:
                                blocks = spec.blocks[gg]
                                pg = gpsum.tile([P, 128], F32, tag="pg")
                                nb = len(blocks)
                                for i, (qq, pos, j) in enumerate(blocks):
                                    if mb_host:
                                        rhs_ap = mb_sb[:, (j - j0) * P:
                                                       (j - j0 + 1) * P]
                                    else:
                                        mb = mpool.tile([P, P], BF16, tag="mb")
                                        nc.vector.tensor_scalar(
                                            mb[:], iota_sb[:], rl_sb[:, j:j + 1],
                                            vv_sb[:, j:j + 1],
                                            mybir.AluOpType.is_equal,
                                            mybir.AluOpType.mult)
                                        rhs_ap = mb[:]
                                    if level < 2.4:
                                        continue
                                    nc.tensor.matmul(pg[:], lhsT=gds[qq][:, pos, :],
                                                     rhs=rhs_ap,
                                                     start=(i == 0),
                                                     stop=(i == nb - 1))
                                if level < 2.4:
                                    continue
                                if level < 2.7:
                                    nc.vector.tensor_copy(
                                        HT[:, gg * P:(gg + 1) * P], pg[:])
                                    continue
                                # PReLU + readout row-sum, on the Activation
                                # engine (one op, keeps DVE free for mb builds)
                                nc.scalar.activation(
                                    HT[:, gg * P:(gg + 1) * P], pg[:],
                                    mybir.ActivationFunctionType.Prelu,
                                    alpha=a_sb[:, 0:1],
                                    accum_out=acc[:, gg:gg + 1])

                # ---------------- phase 3: readout + scores --------------------
                if level >= 4:
                    with (
                        tc.tile_pool(name="ro", bufs=1) as ro,
                        tc.tile_pool(name="rpsum", bufs=1, space="PSUM") as rpsum,
                    ):
                        msum = ro.tile([P, 1], F32)
                        nc.vector.reduce_sum(msum[:], acc[:],
                                             axis=mybir.AxisListType.X)
                        nc.sync.dma_start(ar_in[:], msum[0:64, :])
                        if timing_variant:
                            arb = ro.tile([64, 1], F32)
                            nc.sync.dma_start(arb[:], ar_in[:])
                            nc.sync.dma_start(ar_out[:], arb[:])
                        else:
                            nc.gpsimd.collective_compute(
                                "AllReduce", mybir.AluOpType.add, replica_groups=rg,
                                ins=[ar_in.ap().opt()], outs=[ar_out.ap().opt()],
                            )
                        ssum = ro.tile([64, 1], F32)
                        nc.sync.dma_start(ssum[:], ar_out[:])
                        sig = ro.tile([64, 1], F32)
                        nc.scalar.activation(sig[:], ssum[:],
                                             mybir.ActivationFunctionType.Sigmoid,
                                             scale=inv_n)
                        zp = rpsum.tile([64, 1], F32, tag="zp")
                        nc.tensor.matmul(zp[:], lhsT=wbt_sb[:], rhs=sig[:],
                                         start=True, stop=True)
                        z2 = ro.tile([P, 2], BF16)
                        nc.vector.memset(z2[:], 0.0)
                        nc.scalar.copy(z2[0:64, 0:1], zp[:])
                        nc.scalar.copy(z2[64:128, 1:2], zp[:])
                        sp = rpsum.tile([P, 2 * groups], F32, tag="sp")
                        for t in range(groups):
                            nc.tensor.matmul(sp[:, 2 * t:2 * t + 2],
                                             lhsT=HT[:, t * P:(t + 1) * P],
                                             rhs=z2[:], start=True, stop=True)
                        scr = ro.tile([P, 2 * groups], F32)
                        nc.vector.tensor_scalar_add(scr[:], sp[:], bb_sb[:, 0:1])
                        nc.sync.dma_start(scores_d[:], scr[:])
                else:
                    _early_out(nc, tc, scores_d, groups)

    nc.compile()
    return nc


def _make_in_maps(pl, inputs):
    ncores = pl.ncores
    pos, neg = inputs["pos"], inputs["neg"]
    local_n, local_pad = pl.local_n, pl.local_pad
    a_val = np.float32(np.asarray(inputs["prelu_a"]).reshape(-1)[0])
    bb_val = np.float32(np.asarray(inputs["b_bil"]).reshape(-1)[0])
    iota_bc = np.tile(np.arange(P, dtype=np.float32)[None, :], (P, 1)).astype(
        ml_dtypes.bfloat16)
    w2 = np.ascontiguousarray(
        np.asarray(inputs["W_gcn"]).T.astype(ml_dtypes.bfloat16))
    wbt = np.ascontiguousarray(np.asarray(inputs["W_bil"]).T.astype(np.float32))
    bgv = np.asarray(inputs["b_gcn"]).reshape(-1).astype(np.float32)
    bias2 = np.tile(np.concatenate([bgv, bgv])[None, :], (P, 1)).astype(
        ml_dtypes.bfloat16)

    posT = np.asarray(pos[0]).T.astype(ml_dtypes.bfloat16)   # [128, N]
    negT = np.asarray(neg[0]).T.astype(ml_dtypes.bfloat16)

    in_maps = []
    for c in range(ncores):
        sl = slice(c * local_n, (c + 1) * local_n)
        x2 = np.zeros((P, 2 * local_pad), ml_dtypes.bfloat16)
        x2[:, :local_n] = posT[:, sl]
        x2[:, local_pad:local_pad + local_n] = negT[:, sl]
        mbh = ((pl.rl[c][:, :, None] == np.arange(P, dtype=np.float32)) *
               pl.vv[c][:, :, None]).astype(ml_dtypes.bfloat16).reshape(
                   P, pl.b_total * P)
        in_maps.append({
            "x2": x2,
            "mbh": np.ascontiguousarray(mbh),
            "w2": w2,
            "bias2": bias2,
            "wbt": wbt,
            "acol": np.full((P, 1), a_val, np.float32),
            "bbcol": np.full((P, 1), bb_val, np.float32),
            "iotab": iota_bc,
            "gidx": pl.gidx[c],
            "rl": pl.rl[c],
            "vv": pl.vv[c],
        })
    return in_maps


def _assemble(pl, results, n_total):
    ncores, local_n, local_pad = pl.ncores, pl.local_n, pl.local_pad
    logits = np.zeros((1, 2 * n_total), np.float32)
    for c in range(ncores):
        arr = results[c]["scores"]            # [P, 2*groups]
        posv = arr[:, 0::2].T.reshape(local_pad)[:local_n]
        negv = arr[:, 1::2].T.reshape(local_pad)[:local_n]
        logits[0, c * local_n:(c + 1) * local_n] = posv
        logits[0, n_total + c * local_n:n_total + (c + 1) * local_n] = negv
    return logits


def _run(pos, neg, edge_rows, edge_cols, edge_vals,
         W_gcn, b_gcn, prelu_a, W_bil, b_bil, ncores=8, **run_kwargs):
    n_nodes = pos.shape[1]
    f_dim = pos.shape[2]
    assert f_dim == P

    pl = make_plan(n_nodes, ncores, edge_rows, edge_cols, edge_vals)

    key = (n_nodes, ncores, pl.b_total, pl.gidx_cols,
           tuple(pl.nblk.reshape(-1).tolist()))
    if key in _NC_CACHE:
        nc = _NC_CACHE[key]
    else:
        nc = build_nc(pl)
        _NC_CACHE.clear()
        _NC_CACHE[key] = nc

    in_maps = _make_in_maps(pl, {
        "pos": pos, "neg": neg, "W_gcn": W_gcn, "b_gcn": b_gcn,
        "prelu_a": prelu_a, "W_bil": W_bil, "b_bil": b_bil,
    })

    res = run_bass_kernel_spmd(nc, in_maps, core_ids=list(range(ncores)),
                               **run_kwargs)

    logits = _assemble(pl, res.results, n_nodes)
    return logits, res


def kernel(pos, neg, edge_rows, edge_cols, edge_vals,
           W_gcn, b_gcn, prelu_a, W_bil, b_bil):
    logits, _ = _run(pos, neg, edge_rows, edge_cols, edge_vals,
                     W_gcn, b_gcn, prelu_a, W_bil, b_bil)
    return logits

